# revision 11
# baseline (speedup 1.0000x reference)
"""Trainium2 Bass kernel for nn_BiMultiGCN (bipartite multi-relation LightGCN).

Strategy (8 NeuronCores):
  - 1D-shard nodes: core k owns a contiguous slice of user rows and spot rows.
  - Host prep buckets every edge list by destination 128-node tile inside the
    owning core's slice (pure data movement / sharding, no arithmetic).
  - On device, each pass is: SWDGE dma_gather of source rows (fp16 tables,
    256B rows) -> build one-hot [edge, dst_local] via DVE iota-compare ->
    accumulate segment sums on the tensor engine into PSUM (f32). This avoids
    DMA scatter-add entirely (its CCE add loses updates on duplicate indices
    within one call).
  - Symmetric degree normalization is separable: 1/sqrt(deg_u[u]*deg_s[s]) =
    rsqrt(deg_u[u]) * rsqrt(deg_s[s]). Degrees come from the same one-hot
    matmul with an all-ones rhs; per-node rsqrt scaling is a cheap elementwise
    multiply because the replicated [node,128] degree layout matches the
    embedding tiles.
  - Between layers, each core's freshly produced (already rsqrt-scaled) slice
    is AllGathered so every core can gather from the full node table.
  - int16 gather indices top out at 32767; the spot table (43008 padded rows)
    is gathered in two bank sub-passes (rows <32768 and the rest).
"""

import sys

sys.path.insert(0, "/opt/trn_rl_repo")

import numpy as np

N_USER = 27094
M_SPOT = 42852
H = 128
NLAYERS = 3
NC = 8
BANK = 32768
GPG = 8  # 128-edge groups per chunk (1024 edges/chunk; SWDGE ring caps a gather call at 1024 descriptors)


def _cfg_default():
    return dict(n_user=N_USER, m_spot=M_SPOT, nlayers=NLAYERS, bank=BANK)


def _derive(cfg):
    import math

    ut = math.ceil(cfg["n_user"] / (NC * 128))
    st = math.ceil(cfg["m_spot"] / (NC * 128))
    cfg = dict(cfg)
    cfg["ut"], cfg["st"] = ut, st
    cfg["u_slice"], cfg["s_slice"] = ut * 128, st * 128
    cfg["nu_pad"], cfg["ns_pad"] = NC * ut * 128, NC * st * 128
    return cfg


def _wrap16(a):
    # [C, NE] -> [C, 128, NE//16]; token i -> (partition i%16, col i//16),
    # replicated into all 8 groups of 16 partitions.
    C, NE = a.shape
    x = a.reshape(C, NE // 16, 16).transpose(0, 2, 1)
    return np.tile(x, (1, 8, 1)).astype(np.int16).copy()


def _swz128(a, dt):
    # [C, NE] -> [C, 128, NE//128]; token i -> (partition i%128, col i//128)
    C, NE = a.shape
    return a.reshape(C, NE // 128, 128).transpose(0, 2, 1).astype(dt).copy()


def _flat_bucketed(src, dst, w, ntiles, split, bank):
    """Bucket edges by (dst//128) tile; optional src bank split.

    Returns a list over banks of dicts with per-core padded flat streams:
      gidx [C, NE] int64, dstloc [C, NE] f32 (pad -1), w [C, NE] f32 or None,
      gpt: per-tile group counts (shared across cores).
    """
    n = len(dst)
    tile_g = dst // 128
    if split:
        flag = (src >= bank).astype(np.int64)
        nb = 2
    else:
        flag = np.zeros(n, np.int64)
        nb = 1
    key = tile_g * 2 + flag
    order = np.argsort(key, kind="stable")
    ks = key[order]
    ss = src[order]
    ds = dst[order]
    ws = w[order] if w is not None else None
    ntot = NC * ntiles
    bounds = np.searchsorted(ks, np.arange(2 * ntot + 1))
    out = []
    for bi in range(nb):
        idx2 = 2 * np.arange(ntot) + bi
        cnts = (bounds[idx2 + 1] - bounds[idx2]).reshape(NC, ntiles)
        gpt = np.ceil(cnts.max(0) / 128).astype(np.int64)
        if bi == 0:
            gpt = np.maximum(gpt, 1)
        NEb = int(gpt.sum()) * 128
        gi = np.zeros((NC, NEb), np.int64)
        dl = np.full((NC, NEb), -1.0, np.float32)
        wq = np.zeros((NC, NEb), np.float32) if w is not None else None
        offs = np.concatenate([[0], np.cumsum(gpt) * 128]).astype(np.int64)
        for k in range(NC):
            for t in range(ntiles):
                gt = k * ntiles + t
                lo, hi = bounds[2 * gt + bi], bounds[2 * gt + bi + 1]
                m = hi - lo
                if m == 0:
                    continue
                o = offs[t]
                gi[k, o : o + m] = ss[lo:hi] - (bank if bi == 1 else 0)
                dl[k, o : o + m] = ds[lo:hi] % 128
                if w is not None:
                    wq[k, o : o + m] = ws[lo:hi]
        out.append(dict(gidx=gi, dstloc=dl, w=wq, gpt=[int(x) for x in gpt]))
    return out


def _prepare(inputs, cfg):
    cfg = _derive(cfg)
    ut, st = cfg["ut"], cfg["st"]
    us, ss_ = cfg["u_slice"], cfg["s_slice"]
    bank = cfg["bank"]

    user_emb = np.asarray(inputs["user_emb"], np.float32)
    spot_emb = np.asarray(inputs["spot_emb"], np.float32)
    us_user = np.asarray(inputs["us_user"], np.int64)
    us_spot = np.asarray(inputs["us_spot"], np.int64)

    uemb_pad = np.zeros((cfg["nu_pad"], H), np.float32)
    uemb_pad[: len(user_emb)] = user_emb
    semb_pad = np.zeros((cfg["ns_pad"], H), np.float32)
    semb_pad[: len(spot_emb)] = spot_emb

    passes = {}
    # user-user conv: gather user_emb[src], dst user tiles, weighted
    ue = np.asarray(inputs["user_edge_index"], np.int64)
    passes["uconv"] = _flat_bucketed(
        ue[0], ue[1], np.asarray(inputs["user_edge_weight"], np.float32), ut, False, bank
    )
    # merged spot relation convs (category+city+station): dst spot tiles, weighted
    srcs, dsts, wss = [], [], []
    for name in ("category", "city", "station"):
        ei = np.asarray(inputs[f"{name}_edge_index"], np.int64)
        srcs.append(ei[0])
        dsts.append(ei[1])
        wss.append(np.asarray(inputs[f"{name}_edge_weight"], np.float32))
    passes["sconv"] = _flat_bucketed(
        np.concatenate(srcs), np.concatenate(dsts), np.concatenate(wss), st, True, bank
    )
    # bipartite: user-update (gather spot table, dst user tiles; reused 3 layers)
    passes["bu"] = _flat_bucketed(us_spot, us_user, None, ut, True, bank)
    # bipartite: spot-update (gather user table, dst spot tiles)
    passes["bs"] = _flat_bucketed(us_user, us_spot, None, st, False, bank)
    # degree histogram over us edges bucketed by user tiles (no gather)
    passes["degu"] = _flat_bucketed(np.zeros_like(us_user), us_user, None, ut, False, bank)

    # device input arrays
    arrays = {
        "uemb16": np.broadcast_to(uemb_pad.astype(np.float16), (NC, *uemb_pad.shape)),
        "semb16": np.broadcast_to(semb_pad.astype(np.float16), (NC, *semb_pad.shape)),
        "iota": np.broadcast_to(
            np.tile(np.arange(128, dtype=np.float16), (128, 1)), (NC, 128, 128)
        ),
        "uemb_sl": uemb_pad.reshape(NC, ut, 128, H).transpose(0, 2, 1, 3).reshape(NC, 128, ut * H).copy(),
        "semb_sl": semb_pad.reshape(NC, st, 128, H).transpose(0, 2, 1, 3).reshape(NC, 128, st * H).copy(),
    }
    meta = {"cfg": cfg, "gpt": {}}
    for pname, banks in passes.items():
        for bi, b in enumerate(banks):
            tag = f"{pname}{bi}"
            meta["gpt"][tag] = b["gpt"]
            arrays[f"{tag}_dl"] = _swz128(b["dstloc"], np.float16)
            if pname != "degu":
                arrays[f"{tag}_gi"] = _wrap16(b["gidx"])
            if b["w"] is not None:
                arrays[f"{tag}_w"] = _swz128(b["w"], np.float16)
    return arrays, meta


def _build(meta):
    import concourse.bass as bass
    import concourse.bacc as bacc
    import concourse.mybir as mybir
    from concourse import tile
    from concourse import library_config

    cfg = meta["cfg"]
    ut, st = cfg["ut"], cfg["st"]
    nu_pad, ns_pad = cfg["nu_pad"], cfg["ns_pad"]
    u_slice, s_slice = cfg["u_slice"], cfg["s_slice"]
    bank = cfg["bank"]
    gpt = meta["gpt"]
    f16, f32, i16 = mybir.dt.float16, mybir.dt.float32, mybir.dt.int16
    MUL = mybir.AluOpType.mult
    ADD = mybir.AluOpType.add
    EQ = mybir.AluOpType.is_equal

    nc = bacc.Bacc("TRN2", target_bir_lowering=False, debug=False, num_devices=NC)

    def din(name, shape, dt):
        return nc.dram_tensor(name, shape, dt, kind="ExternalInput")

    t_uemb16 = din("uemb16", [nu_pad, H], f16)
    t_semb16 = din("semb16", [ns_pad, H], f16)
    t_iota = din("iota", [128, 128], f16)
    t_uemb_sl = din("uemb_sl", [128, ut * H], f32)
    t_semb_sl = din("semb_sl", [128, st * H], f32)
    t_in = {}
    for tag, g in gpt.items():
        ne = sum(g) * 128
        t_in[f"{tag}_dl"] = din(f"{tag}_dl", [128, ne // 128], f16)
        if not tag.startswith("degu"):
            t_in[f"{tag}_gi"] = din(f"{tag}_gi", [128, ne // 16], i16)
        if tag.startswith(("uconv", "sconv")):
            t_in[f"{tag}_w"] = din(f"{tag}_w", [128, ne // 128], f16)

    t_out_u = nc.dram_tensor("out_user", [128, ut * H], f32, kind="ExternalOutput")
    t_out_s = nc.dram_tensor("out_spot", [128, st * H], f32, kind="ExternalOutput")

    # internal DRAM
    import kernel as _KK
    if _KK.DBG_BOUNCE:
        uemb_b = nc.dram_tensor("uemb_b", [nu_pad, H], f16)
        semb_b = nc.dram_tensor("semb_b", [ns_pad, H], f16)
    yu_slice = nc.dram_tensor("yu_slice", [u_slice, H], f16)
    ys_slice = nc.dram_tensor("ys_slice", [s_slice, H], f16)
    yu_full = [nc.dram_tensor(f"yu_full{s}", [nu_pad, H], f16, addr_space="Shared") for s in range(NLAYERS)]
    ys_full = [nc.dram_tensor(f"ys_full{s}", [ns_pad, H], f16, addr_space="Shared") for s in range(NLAYERS)]

    with tile.TileContext(nc) as tc:
        nc.gpsimd.load_library(library_config.mlp)
        with (
            tc.tile_pool(name="res", bufs=1) as res,
            tc.tile_pool(name="ck", bufs=3) as ck,
            tc.tile_pool(name="ep", bufs=3) as ep,
            tc.tile_pool(name="ps", bufs=4, space="PSUM") as ps,
        ):
            inv_u = res.tile([128, ut * H], f32, tag="inv_u")
            inv_s = res.tile([128, st * H], f32, tag="inv_s")
            user_out = res.tile([128, ut * H], f32, tag="user_out")
            spot_out = res.tile([128, st * H], f32, tag="spot_out")
            raw_u = res.tile([128, ut * H], f32, tag="raw_u")
            raw_s = res.tile([128, st * H], f32, tag="raw_s")
            iota16 = res.tile([128, 128], f16, tag="iota16")
            ones16 = res.tile([128, 128], f16, tag="ones16")
            nc.gpsimd.dma_start(iota16[:], t_iota[:])
            nc.vector.memset(ones16[:], 1.0)
            if _KK.DBG_BOUNCE:
                nc.gpsimd.dma_start(uemb_b[:, :], t_uemb16[:, :])
                nc.gpsimd.dma_start(semb_b[:, :], t_semb16[:, :])
                uemb_g, semb_g = uemb_b, semb_b
            else:
                uemb_g, semb_g = t_uemb16, t_semb16

            def seg_pass(tag, gather_ap, weighted, epilogue):
                """One segment-sum pass: per dst tile accumulate
                sum_e onehot[e,dst] (x) rhs[e,:] into PSUM, then epilogue(t, acc)."""
                g_list = gpt[tag]
                G = sum(g_list)
                nch = (G + GPG - 1) // GPG
                # group -> (tile, j, n) map
                gmap = []
                for t, n in enumerate(g_list):
                    for j in range(n):
                        gmap.append((t, j, n))
                t_dl = t_in[f"{tag}_dl"]
                t_gi = t_in.get(f"{tag}_gi")
                t_w = t_in.get(f"{tag}_w")
                acc = None
                done = [False] * len(g_list)
                for c in range(nch):
                    g0 = c * GPG
                    ng = min(GPG, G - g0)
                    ne_c = ng * 128
                    dl = ck.tile([128, GPG], f16, tag="dl")
                    nc.gpsimd.dma_start(dl[:, :ng], t_dl[:, g0 : g0 + ng])
                    oh = ck.tile([128, GPG, 128], f16, tag="oh")
                    b1, b2 = bass.broadcast_tensor_aps(dl[:, :ng, None], iota16[:, None, :])
                    nc.vector.tensor_tensor(oh[:, :ng, :], b1, b2, EQ)
                    if gather_ap is not None:
                        msgs = ck.tile([128, GPG, H], f16, tag="msgs")
                        if _K.DBG_NOGATHER:
                            nc.vector.memset(msgs[:, :ng, :], 0.5)
                        else:
                            gi = ck.tile([128, GPG * 8], i16, tag="gi")
                            nc.gpsimd.dma_start(gi[:, : ne_c // 16], t_gi[:, g0 * 8 : g0 * 8 + ne_c // 16])
                            nc.gpsimd.dma_gather(msgs[:, :ng, :], gather_ap, gi[:, : ne_c // 16], ne_c, ne_c, H)
                        if weighted and not _K.DBG_NOW:
                            wt = ck.tile([128, GPG], f16, tag="wt")
                            nc.gpsimd.dma_start(wt[:, :ng], t_w[:, g0 : g0 + ng])
                            a1, a2 = bass.broadcast_tensor_aps(msgs[:, :ng, :], wt[:, :ng, None])
                            nc.vector.tensor_tensor(msgs[:, :ng, :], a1, a2, MUL)
                    else:
                        msgs = None
                    for s in range(ng):
                        t, j, n = gmap[g0 + s]
                        if j == 0:
                            acc = ps.tile([128, H], f32, tag="acc")
                        rhs = msgs[:, s, :] if msgs is not None else ones16[:]
                        if not _K.DBG_NOMM:
                            nc.tensor.matmul(acc[:], oh[:, s, :], rhs, start=(j == 0), stop=(j == n - 1))
                        elif j == 0:
                            nc.vector.memset(acc[:], 0.0)
                        if j == n - 1:
                            epilogue(t, acc)
                            done[t] = True
                for t, fin in enumerate(done):
                    if not fin:
                        epilogue(t, None)

            def cols(t):
                return slice(t * H, (t + 1) * H)

            # ---- degree passes -> inv_u / inv_s -------------------------------
            def ep_deg(inv):
                def f(t, acc):
                    if acc is None:
                        nc.vector.memset(inv[:, cols(t)], 1.0)
                        return
                    tmp = ep.tile([128, H], f32, tag="tmp")
                    nc.vector.tensor_scalar_max(tmp[:], acc[:], 1.0)
                    sq = ep.tile([128, H], f32, tag="sq")
                    nc.scalar.activation(sq[:], tmp[:], mybir.ActivationFunctionType.Sqrt)
                    nc.vector.reciprocal(inv[:, cols(t)], sq[:])
                return f

            import kernel as _K
            _STG = _K.STAGES
            import os as _os
            _SSET = _os.environ.get("K_STAGESET")
            def _on(n, _STG=_STG, _SSET=_SSET):
                if _SSET is not None:
                    return str(n) in _SSET.split(",")
                return _STG >= n
            if _on(1):
                seg_pass("degu0", None, False, ep_deg(inv_u))
                seg_pass("bs0", None, False, ep_deg(inv_s))
            else:
                nc.vector.memset(inv_u[:], 1.0)
                nc.vector.memset(inv_s[:], 1.0)

            # ---- user conv -> user_out (=user_x), yu ---------------------------
            def ep_uconv(t, acc):
                if _K.DBG_NOEPC:
                    return
                if _K.DBG_NOEP:
                    if acc is not None:
                        nc.vector.tensor_copy(user_out[:, cols(t)], acc[:])
                    return
                emb = ep.tile([128, H], f32, tag="emb")
                nc.gpsimd.dma_start(emb[:], t_uemb_sl[:, cols(t)])
                x = user_out[:, cols(t)]
                if acc is None:
                    nc.vector.tensor_copy(x, emb[:])
                else:
                    nc.vector.tensor_add(x, emb[:], acc[:])
                y16 = ep.tile([128, H], f16, tag="y16")
                nc.vector.tensor_tensor(y16[:], x, inv_u[:, cols(t)], MUL)
                nc.gpsimd.dma_start(yu_slice[t * 128 : (t + 1) * 128, :], y16[:])

            if _on(2):
                seg_pass("uconv0", uemb_g[:, :], True, ep_uconv)
            else:
                nc.vector.memset(user_out[:], 0.0)
            if _on(3):
                nc.gpsimd.collective_compute(
                "AllGather", mybir.AluOpType.bypass, replica_groups=[list(range(NC))],
                    ins=[yu_slice[:, :]], outs=[yu_full[0][:, :]],
                )

            # ---- spot conv (two banks) -> spot_out (=spot_x), ys ---------------
            def ep_sconvA(t, acc):
                if acc is None:
                    nc.vector.memset(raw_s[:, cols(t)], 0.0)
                else:
                    nc.vector.tensor_copy(raw_s[:, cols(t)], acc[:])

            if _on(4):
                seg_pass("sconv0", semb_g[0:bank, :], True, ep_sconvA)
            else:
                nc.vector.memset(raw_s[:], 0.0)

            def ep_sconvB(t, acc):
                emb = ep.tile([128, H], f32, tag="emb")
                nc.gpsimd.dma_start(emb[:], t_semb_sl[:, cols(t)])
                x = spot_out[:, cols(t)]
                if acc is None:
                    tot = raw_s[:, cols(t)]
                else:
                    tot = ep.tile([128, H], f32, tag="tmp")
                    nc.vector.tensor_add(tot[:], raw_s[:, cols(t)], acc[:])
                    tot = tot[:]
                nc.vector.scalar_tensor_tensor(x, tot, 1.0 / 3.0, emb[:], op0=MUL, op1=ADD)
                y16 = ep.tile([128, H], f16, tag="y16")
                nc.vector.tensor_tensor(y16[:], x, inv_s[:, cols(t)], MUL)
                nc.gpsimd.dma_start(ys_slice[t * 128 : (t + 1) * 128, :], y16[:])

            if _on(5):
                seg_pass("sconv1", semb_g[bank:ns_pad, :], True, ep_sconvB)
            else:
                nc.vector.memset(spot_out[:], 0.0)
            if _on(6):
                nc.gpsimd.collective_compute(
                "AllGather", mybir.AluOpType.bypass, replica_groups=[list(range(NC))],
                    ins=[ys_slice[:, :]], outs=[ys_full[0][:, :]],
                )

            # ---- bipartite layers ---------------------------------------------
            for L in range(NLAYERS if _on(7) else 0):
                last = L == NLAYERS - 1
                ys_prev, yu_prev = ys_full[L], yu_full[L]

                def ep_buA(t, acc):
                    if acc is None:
                        nc.vector.memset(raw_u[:, cols(t)], 0.0)
                    else:
                        nc.vector.tensor_copy(raw_u[:, cols(t)], acc[:])

                seg_pass("bu0", ys_prev[0:bank, :], False, ep_buA)

                def ep_buB(t, acc, last=last):
                    if acc is None:
                        tot = raw_u[:, cols(t)]
                    else:
                        tot = ep.tile([128, H], f32, tag="tmp")
                        nc.vector.tensor_add(tot[:], raw_u[:, cols(t)], acc[:])
                        tot = tot[:]
                    new = ep.tile([128, H], f32, tag="new")
                    nc.vector.tensor_tensor(new[:], tot, inv_u[:, cols(t)], MUL)
                    nc.vector.tensor_add(user_out[:, cols(t)], user_out[:, cols(t)], new[:])
                    if not last:
                        y16 = ep.tile([128, H], f16, tag="y16")
                        nc.vector.tensor_tensor(y16[:], new[:], inv_u[:, cols(t)], MUL)
                        nc.gpsimd.dma_start(yu_slice[t * 128 : (t + 1) * 128, :], y16[:])

                seg_pass("bu1", ys_prev[bank:ns_pad, :], False, ep_buB)

                def ep_bs(t, acc, last=last):
                    new = ep.tile([128, H], f32, tag="new")
                    if acc is None:
                        nc.vector.memset(new[:], 0.0)
                    else:
                        nc.vector.tensor_tensor(new[:], acc[:], inv_s[:, cols(t)], MUL)
                    nc.vector.tensor_add(spot_out[:, cols(t)], spot_out[:, cols(t)], new[:])
                    if not last:
                        y16 = ep.tile([128, H], f16, tag="y16")
                        nc.vector.tensor_tensor(y16[:], new[:], inv_s[:, cols(t)], MUL)
                        nc.gpsimd.dma_start(ys_slice[t * 128 : (t + 1) * 128, :], y16[:])

                seg_pass("bs0", yu_prev[0:nu_pad, :], False, ep_bs)

                if not last:
                    nc.gpsimd.collective_compute(
                        "AllGather", mybir.AluOpType.bypass, replica_groups=[list(range(NC))],
                        ins=[yu_slice[:, :]], outs=[yu_full[L + 1][:, :]],
                    )
                    nc.gpsimd.collective_compute(
                        "AllGather", mybir.AluOpType.bypass, replica_groups=[list(range(NC))],
                        ins=[ys_slice[:, :]], outs=[ys_full[L + 1][:, :]],
                    )

            # ---- outputs -------------------------------------------------------
            if not _K.DBG_NOFINAL:
                sc = 1.0 / (NLAYERS + 1)
                nc.scalar.mul(user_out[:], user_out[:], sc)
                nc.scalar.mul(spot_out[:], spot_out[:], sc)
                nc.gpsimd.dma_start(t_out_u[:], user_out[:])
                nc.gpsimd.dma_start(t_out_s[:], spot_out[:])

    nc.compile()
    return nc


_LAST = {}
STAGES = 99  # debug: limit how many build stages are emitted
DBG_NOGATHER = False
DBG_NOW = False
DBG_NOEP = False
DBG_NOEPC = False
DBG_NOFINAL = False
DBG_NOMM = False
DBG_BOUNCE = False


def _execute(nc, arrays, meta, trace=False):
    from concourse.bass_utils import run_bass_kernel_spmd

    in_maps = []
    for k in range(NC):
        m = {}
        for name, arr in arrays.items():
            m[name] = np.ascontiguousarray(arr[k])
        in_maps.append(m)
    return run_bass_kernel_spmd(nc, in_maps, list(range(NC)), trace=trace)


def _assemble(res, meta):
    cfg = meta["cfg"]
    ut, st = cfg["ut"], cfg["st"]
    outs_u, outs_s = [], []
    for k in range(NC):
        ou = res.results[k]["out_user"].reshape(128, ut, H).transpose(1, 0, 2).reshape(ut * 128, H)
        os_ = res.results[k]["out_spot"].reshape(128, st, H).transpose(1, 0, 2).reshape(st * 128, H)
        outs_u.append(ou)
        outs_s.append(os_)
    user = np.concatenate(outs_u)[: cfg["n_user"]]
    spot = np.concatenate(outs_s)[: cfg["m_spot"]]
    return spot.astype(np.float32), user.astype(np.float32)


def run(inputs, cfg=None):
    cfg = cfg or _cfg_default()
    arrays, meta = _prepare(inputs, cfg)
    nc = _build(meta)
    res = _execute(nc, arrays, meta)
    _LAST.update(nc=nc, arrays=arrays, meta=meta, res=res)
    return _assemble(res, meta)


def kernel(**inputs):
    return run(inputs)
